# revision 10
# baseline (speedup 1.0000x reference)
"""Trainium2 Bass kernel for nn_MultiHeadAttention (B=2, S=2048, D=512, H=8).

Sharding: 8 cores = 2 batches x 4 head-pairs. Core c handles batch b=c//4 and
heads {hp, hp+4} with hp=c%4 (the pair shares rope frequencies and the same
128 columns of the QK projections, so the projection work is shared).

Per-core device pipeline (all seq-transposed, "layout B" — sk on partitions):
  f32r projections (X^T @ W slices, plus half-swapped copies so rope is
  partition-aligned) -> rope on DVE (fp16 q^T/k^T)
  -> logits^T = k @ q^T per head (fp16 matmuls, fp32 PSUM)
  -> exp on ScalarE (scale=1/8 folded in, fp16 out, FD=1024 ops)
  -> V-matmul with a ones-column appended to v => embeds^T and softmax
     denominators accumulate in one PSUM tile
  -> reciprocal + PE rank-1 broadcast -> DVE normalize -> fp16 scores^T out
  -> output projection partials (E^T @ Wo^T slice) -> fp16 partial out
Host side: shard prep (transpose/permute/cast) and unshard (transpose fp16
scores^T back to (sq, sk) f32, sum the 4 per-batch output-projection partials).
"""

import numpy as np

B, S, D = 2, 2048, 512
NUM_HEADS, HEAD_DIM = 8, 64
P = 128          # partitions
NT = 512         # free-dim tile (one PSUM bank of f32)
NT2 = 1024       # doubled free-dim tile for ACT exp ops
NKC = D // P     # 4 contraction chunks for projections
NSK = S // P     # 16 sk chunks
NSK2 = NSK // 2  # 8 pairs of sk chunks
NSQ = S // NT    # 4 sq tiles
VW = 2 * (HEAD_DIM + 1)  # 130: v columns per sk chunk (2 heads + 2 ones cols)

_CACHE = {}


def _build_nc():
    import concourse.tile as tile
    import concourse.mybir as mybir
    from concourse import bacc
    from contextlib import ExitStack

    f32 = mybir.dt.float32
    f32r = mybir.dt.float32r
    f16 = mybir.dt.float16
    Exp = mybir.ActivationFunctionType.Exp

    nc = bacc.Bacc("TRN2", target_bir_lowering=False, debug=False, num_devices=8)

    xqt = nc.dram_tensor("xqt", [D, S], f16, kind="ExternalInput").ap()
    xkt = nc.dram_tensor("xkt", [D, S], f16, kind="ExternalInput").ap()
    xvt = nc.dram_tensor("xvt", [D, S], f16, kind="ExternalInput").ap()
    wpack = nc.dram_tensor("wpack", [D, 5 * P], f16, kind="ExternalInput").ap()
    wo = nc.dram_tensor("wo", [P, D], f16, kind="ExternalInput").ap()
    cs = nc.dram_tensor("cs", [P, 2 * S], f16, kind="ExternalInput").ap()

    s_out = [
        nc.dram_tensor(f"s{h}", [S, S], f16, kind="ExternalOutput").ap()
        for h in range(2)
    ]
    p_out = nc.dram_tensor("po", [S, D], f16, kind="ExternalOutput").ap()

    with tile.TileContext(nc) as tc:
        with ExitStack() as ctx:
            consts = ctx.enter_context(tc.tile_pool(name="consts", bufs=1))
            xpool = ctx.enter_context(tc.tile_pool(name="xp", bufs=10))
            qkpool = ctx.enter_context(tc.tile_pool(name="qk", bufs=1))
            exps = ctx.enter_context(tc.tile_pool(name="exps", bufs=10))
            norms = ctx.enter_context(tc.tile_pool(name="norms", bufs=2))
            small = ctx.enter_context(tc.tile_pool(name="small", bufs=2))
            stage = ctx.enter_context(tc.tile_pool(name="stage", bufs=3))
            ps_l = ctx.enter_context(tc.tile_pool(name="psl", bufs=2, space="PSUM"))
            ps_e = ctx.enter_context(tc.tile_pool(name="pse", bufs=2, space="PSUM"))
            ps_m = ctx.enter_context(tc.tile_pool(name="psm", bufs=2, space="PSUM"))

            # warm the ACT exp table while DMAs stream in
            warm = consts.tile([1, 8], f32, tag="warm")
            nc.vector.memset(warm, 0.0)
            warm16 = consts.tile([1, 8], f16, tag="warm16")
            nc.scalar.activation(warm16, warm, Exp, scale=1.0)

            # ---- constants (one packed DMA each; X inputs on the ACT queue) ----
            WPW = 5 * P
            w_pack_sb = consts.tile([P, NKC * WPW], f16, tag="wpk")
            nc.sync.dma_start(
                out=w_pack_sb.rearrange("p (kc w) -> p kc w", w=WPW),
                in_=wpack.rearrange("(kc p) w -> p kc w", p=P),
            )
            w_off = {"wq": 0, "wk": P, "wq2": 2 * P, "wk2": 3 * P, "wv": 4 * P}

            def w_sl(name, kc):
                return w_pack_sb[:, kc * WPW + w_off[name]:kc * WPW + w_off[name] + P]

            wo_sb = [None, None]
            for h in range(2):
                t = consts.tile([HEAD_DIM, D], f16, tag=f"wo{h}", name=f"wo{h}")
                nc.sync.dma_start(
                    out=t, in_=wo[h * HEAD_DIM:(h + 1) * HEAD_DIM, :]
                )
                wo_sb[h] = t
            cs_sb = consts.tile([P, 2 * S], f16, tag="cs")
            nc.sync.dma_start(out=cs_sb, in_=cs)
            cos_sb = cs_sb[:, 0:S]
            sin_sb = cs_sb[:, S:2 * S]
            ones_sb = consts.tile([1, P], f16, tag="ones")
            nc.vector.memset(ones_sb, 1.0)

            x_sb = {}
            for name, dram, eng in (
                ("xk", xkt, nc.scalar), ("xq", xqt, nc.scalar), ("xv", xvt, nc.sync)
            ):
                t = consts.tile([P, NKC * S], f16, tag=f"x_{name}")
                for kc in range(NKC):
                    eng.dma_start(
                        out=t[:, kc * S:(kc + 1) * S],
                        in_=dram[kc * P:(kc + 1) * P, :],
                    )
                x_sb[name] = t

            def x_sl(name, kc, lo, hi):
                return x_sb[name][:, kc * S + lo:kc * S + hi]

            # ---- phase 1: projections + rope ----
            qT = qkpool.tile([P, S], f16, tag="qT")
            kT = qkpool.tile([P, S], f16, tag="kT")
            v_all = qkpool.tile([P, NSK * VW], f16, tag="v")
            # ones columns of v (cols 64 and 129 of each 130-block)
            nc.vector.memset(
                v_all.rearrange("p (c t) -> p c t", t=HEAD_DIM + 1)[:, :, HEAD_DIM:],
                1.0,
            )

            def proj_rope(wname, xname, dst, ntile):
                sl = slice(ntile * NT, (ntile + 1) * NT)
                py = ps_m.tile([P, NT], f32, tag="psm")
                py2 = ps_m.tile([P, NT], f32, tag="psm")
                for kc in range(NKC):
                    nc.tensor.matmul(
                        py,
                        lhsT=w_sl(wname, kc),
                        rhs=x_sl(xname, kc, ntile * NT, (ntile + 1) * NT),
                        start=(kc == 0),
                        stop=(kc == NKC - 1),
                    )
                for kc in range(NKC):
                    nc.tensor.matmul(
                        py2,
                        lhsT=w_sl(wname + "2", kc),
                        rhs=x_sl(xname, kc, ntile * NT, (ntile + 1) * NT),
                        start=(kc == 0),
                        stop=(kc == NKC - 1),
                    )
                # rope: dst = y*cos + y_swapped*(signed sin), all aligned
                t = small.tile([P, NT], f32, tag="ropet", bufs=3)
                u = small.tile([P, NT], f32, tag="ropeu", bufs=3)
                nc.vector.tensor_mul(t, py, cos_sb[:, sl])
                nc.vector.tensor_mul(u, py2, sin_sb[:, sl])
                nc.gpsimd.tensor_add(dst[:, sl], t, u)

            for ntile in range(NSQ):
                proj_rope("wk", "xk", kT, ntile)
            proj_rope("wq", "xq", qT, 0)

            # v: direct orientation (sk on partitions)
            for c in range(NSK):
                pv = ps_m.tile([P, P], f32, tag="psm")
                for kc in range(NKC):
                    nc.tensor.matmul(
                        pv,
                        lhsT=x_sl("xv", kc, c * P, (c + 1) * P),
                        rhs=w_sl("wv", kc),
                        start=(kc == 0),
                        stop=(kc == NKC - 1),
                    )
                nc.vector.tensor_copy(
                    v_all[:, c * VW:c * VW + HEAD_DIM], pv[:, 0:HEAD_DIM]
                )
                nc.vector.tensor_copy(
                    v_all[:, c * VW + HEAD_DIM + 1:c * VW + 2 * HEAD_DIM + 1],
                    pv[:, HEAD_DIM:2 * HEAD_DIM],
                )

            # ---- phase 2: attention, interleaving heads per sq tile ----
            e_sb = [
                qkpool.tile([HEAD_DIM, S], f16, tag=f"e{h}", name=f"e{h}")
                for h in range(2)
            ]
            for sq_t in range(NSQ):
                if sq_t > 0:
                    proj_rope("wq", "xq", qT, sq_t)
                sq = slice(sq_t * NT, (sq_t + 1) * NT)
                for h in range(2):
                    po = 64 * h
                    vo = (HEAD_DIM + 1) * h
                    pe_acc = ps_e.tile([HEAD_DIM + 1, NT], f32, tag="pse")
                    etiles = []
                    for c2 in range(NSK2):
                        pl = ps_l.tile([P, NT2], f32, tag="psl")
                        for half in range(2):
                            c = 2 * c2 + half
                            nc.tensor.matmul(
                                pl[:, half * NT:(half + 1) * NT],
                                lhsT=kT[po:po + 64, c * P:(c + 1) * P],
                                rhs=qT[po:po + 64, sq],
                                start=True,
                                stop=True,
                            )
                        e = exps.tile([P, NT2], f16, tag="exp")
                        nc.scalar.activation(e, pl, Exp, scale=0.125)
                        etiles.append(e)
                    for c2 in range(NSK2):
                        for half in range(2):
                            c = 2 * c2 + half
                            nc.tensor.matmul(
                                pe_acc[0:HEAD_DIM + 1, :],
                                lhsT=v_all[
                                    :, c * VW + vo:c * VW + vo + HEAD_DIM + 1
                                ],
                                rhs=etiles[c2][:, half * NT:(half + 1) * NT],
                                start=(c == 0),
                                stop=(c == NSK - 1),
                            )
                    # denominators -> reciprocal -> fp16 row
                    dn = small.tile([1, NT], f32, tag="dn")
                    nc.vector.tensor_copy(dn, pe_acc[HEAD_DIM:HEAD_DIM + 1, :])
                    rc = small.tile([1, NT], f32, tag="rc")
                    nc.vector.reciprocal_approx_fast(out=rc, in_=dn)
                    rc16 = small.tile([1, NT], f16, tag="rc16")
                    nc.vector.tensor_copy(rc16, rc)
                    # broadcast across 128 partitions via rank-1 matmul
                    pb = ps_m.tile([P, NT], f32, tag="psm")
                    nc.tensor.matmul(pb, lhsT=ones_sb, rhs=rc16, start=True, stop=True)
                    bc = small.tile([P, NT], f16, tag="bc")
                    nc.vector.tensor_copy(bc, pb)
                    # normalize scores + batched DMA out
                    nall = norms.tile([P, NSK * NT], f16, tag="norm")
                    for c2 in range(NSK2):
                        for half in range(2):
                            c = 2 * c2 + half
                            nc.vector.tensor_mul(
                                nall[:, c * NT:(c + 1) * NT],
                                etiles[c2][:, half * NT:(half + 1) * NT],
                                bc,
                            )
                    last = (sq_t == NSQ - 1) and (h == 1)
                    if last:
                        sv = s_out[h].rearrange(
                            "(g c p) (t x) -> p g c t x", p=P, c=NSQ, x=NT
                        )
                        nv = nall.rearrange(
                            "p (g c o x) -> p g c o x", c=NSQ, o=1, x=NT
                        )
                        for g in range(NSK // NSQ):
                            nc.sync.dma_start(
                                out=sv[:, g:g + 1, :, sq_t:sq_t + 1, :],
                                in_=nv[:, g:g + 1, :, :, :],
                            )
                    else:
                        nc.sync.dma_start(
                            out=s_out[h].rearrange(
                                "(c p) (t x) -> p c t x", p=P, x=NT
                            )[:, :, sq_t:sq_t + 1, :],
                            in_=nall.rearrange("p (c o x) -> p c o x", o=1, x=NT),
                        )
                    # normalize embeds slice
                    nc.vector.tensor_mul(
                        e_sb[h][:, sq], pe_acc[0:HEAD_DIM, :], bc[0:HEAD_DIM, :]
                    )
                # output projection for the 4 sq chunks of this sq tile
                stq = stage.tile([P, NSQ * D], f16, tag="st")
                for ci in range(NT // P):
                    sc = sq_t * (NT // P) + ci
                    po_ps = ps_m.tile([P, D], f32, tag="psm")
                    nc.tensor.matmul(
                        po_ps,
                        lhsT=e_sb[0][:, sc * P:(sc + 1) * P],
                        rhs=wo_sb[0],
                        start=True,
                        stop=False,
                    )
                    nc.tensor.matmul(
                        po_ps,
                        lhsT=e_sb[1][:, sc * P:(sc + 1) * P],
                        rhs=wo_sb[1],
                        start=False,
                        stop=True,
                    )
                    nc.scalar.copy(stq[:, ci * D:(ci + 1) * D], po_ps)
                nc.sync.dma_start(
                    out=p_out.rearrange(
                        "(t ci p) d -> p t ci d", p=P, ci=NSQ
                    )[:, sq_t:sq_t + 1, :, :],
                    in_=stq.rearrange("p (o ci d) -> p o ci d", o=1, d=D),
                )

    nc.compile()
    return nc


def _get_nc():
    if "nc" not in _CACHE:
        _CACHE["nc"] = _build_nc()
    return _CACHE["nc"]


def _rope_tables(hp):
    """cos/sin tiles (128, S) f32 for head pair (hp, hp+4), doubled rows;
    sin rows carry the rope signs (-sin on top half, +sin on bottom)."""
    freqs = np.exp(
        np.arange(0, D, 2, dtype=np.float32) * np.float32(-np.log(10000.0) / D)
    ).astype(np.float32)
    sel = freqs[64 * hp:64 * hp + 64]
    pos = np.arange(S, dtype=np.float32)
    ang = pos[:, None] * sel[None, :]          # (S, 64) f32
    ct = np.cos(ang).astype(np.float32).T      # (64, S)
    st = np.sin(ang).astype(np.float32).T
    c = np.ascontiguousarray(np.concatenate([ct, ct], axis=0))
    s = np.ascontiguousarray(np.concatenate([-st, st], axis=0))
    return c, s


def _transpose_f16_to_f32(a):
    """(S, S) fp16 -> transposed (S, S) fp32, cache-blocked."""
    out = np.empty((S, S), np.float32)
    blk = 128
    for j in range(0, S, blk):
        out[:, j:j + blk] = a[j:j + blk, :].T.astype(np.float32)
    return out


def _numpy_reference(querys, keys, values, attn_mask, Wq, Wk, Wv, Wo):
    def rope(x):
        b, s, d = x.shape
        pos = np.arange(s, dtype=np.float32)[:, None]
        freqs = np.exp(
            np.arange(0, d, 2, dtype=np.float32) * np.float32(-np.log(10000.0) / d)
        )
        ang = pos * freqs
        cos = np.cos(ang)[None].astype(np.float32)
        sin = np.sin(ang)[None].astype(np.float32)
        xe, xo = x[..., 0::2], x[..., 1::2]
        return np.concatenate([xe * cos - xo * sin, xo * cos + xe * sin], axis=-1)

    def split_heads(x):
        b, s, d = x.shape
        return x.reshape(b, s, NUM_HEADS, HEAD_DIM).transpose(0, 2, 1, 3)

    q = split_heads(rope(querys @ Wq.T))
    k = split_heads(rope(keys @ Wk.T))
    v = split_heads(values @ Wv.T)
    logits = np.einsum("bhqd,bhkd->bhqk", q, k)
    logits = logits + attn_mask[:, :, :S, :S] * np.float32(-1e9)
    x = logits / np.float32(np.sqrt(HEAD_DIM))
    x = x - x.max(axis=-1, keepdims=True)
    ex = np.exp(x)
    attn = ex / ex.sum(axis=-1, keepdims=True)
    emb = np.einsum("bhqk,bhkd->bhqd", attn, v)
    b, h, s, hd = emb.shape
    emb = emb.transpose(0, 2, 1, 3).reshape(b, s, h * hd) @ Wo.T
    return emb.astype(np.float32), attn.astype(np.float32)


def kernel(querys, keys, values, attn_mask, Wq, Wk, Wv, Wo):
    querys = np.asarray(querys, dtype=np.float32)
    keys = np.asarray(keys, dtype=np.float32)
    values = np.asarray(values, dtype=np.float32)
    attn_mask = np.asarray(attn_mask, dtype=np.float32)
    Wq = np.asarray(Wq, dtype=np.float32)
    Wk = np.asarray(Wk, dtype=np.float32)
    Wv = np.asarray(Wv, dtype=np.float32)
    Wo = np.asarray(Wo, dtype=np.float32)

    if np.any(attn_mask):
        # The device kernel hardcodes a zero mask (spec fill: zeros).
        return _numpy_reference(
            querys, keys, values, attn_mask, Wq, Wk, Wv, Wo
        )

    from concourse.bass_utils import run_bass_kernel_spmd

    nc = _get_nc()

    xt = {}
    for b in range(B):
        xt[("q", b)] = np.ascontiguousarray(querys[b].T).astype(np.float16)
        xt[("k", b)] = np.ascontiguousarray(keys[b].T).astype(np.float16)
        xt[("v", b)] = np.ascontiguousarray(values[b].T).astype(np.float16)

    in_maps = []
    for c in range(8):
        b, hp = divmod(c, 4)
        perm_qk = np.concatenate(
            [128 * hp + 2 * np.arange(64), 128 * hp + 2 * np.arange(64) + 1]
        )
        perm_qk2 = np.concatenate(
            [128 * hp + 2 * np.arange(64) + 1, 128 * hp + 2 * np.arange(64)]
        )
        perm_v = np.concatenate(
            [64 * hp + np.arange(64), 256 + 64 * hp + np.arange(64)]
        )
        cos_t, sin_t = _rope_tables(hp)
        in_maps.append({
            "xqt": xt[("q", b)],
            "xkt": xt[("k", b)],
            "xvt": xt[("v", b)],
            "wpack": np.ascontiguousarray(np.concatenate([
                Wq[perm_qk, :].T, Wk[perm_qk, :].T,
                Wq[perm_qk2, :].T, Wk[perm_qk2, :].T,
                Wv[perm_v, :].T,
            ], axis=1)).astype(np.float16),
            "wo": np.ascontiguousarray(Wo[:, perm_v].T).astype(np.float16),
            "cs": np.ascontiguousarray(
                np.concatenate([cos_t, sin_t], axis=1)
            ).astype(np.float16),
        })

    res = run_bass_kernel_spmd(nc, in_maps, core_ids=list(range(8)))

    attn_scores = np.empty((B, NUM_HEADS, S, S), np.float32)
    embeds = np.zeros((B, S, D), np.float32)
    for c in range(8):
        b, hp = divmod(c, 4)
        attn_scores[b, hp] = _transpose_f16_to_f32(res.results[c]["s0"])
        attn_scores[b, hp + 4] = _transpose_f16_to_f32(res.results[c]["s1"])
        embeds[b] += res.results[c]["po"].astype(np.float32)

    return embeds, attn_scores


# revision 11
# speedup vs baseline: 1.0543x; 1.0543x over previous
"""Trainium2 Bass kernel for nn_MultiHeadAttention (B=2, S=2048, D=512, H=8).

Sharding: 8 cores = 2 batches x 4 head-pairs. Core c handles batch b=c//4 and
heads {hp, hp+4} with hp=c%4 (the pair shares rope frequencies and the same
128 columns of the QK projections, so the projection work is shared).

Per-core device pipeline (all seq-transposed, "layout B" — sk on partitions):
  f32r projections (X^T @ W slices, plus half-swapped copies so rope is
  partition-aligned) -> rope on DVE (fp16 q^T/k^T)
  -> logits^T = k @ q^T per head (fp16 matmuls, fp32 PSUM)
  -> exp on ScalarE (scale=1/8 folded in, fp16 out, FD=1024 ops)
  -> V-matmul with a ones-column appended to v => embeds^T and softmax
     denominators accumulate in one PSUM tile
  -> reciprocal + PE rank-1 broadcast -> DVE normalize -> fp16 scores^T out
  -> output projection partials (E^T @ Wo^T slice) -> fp16 partial out
Host side: shard prep (transpose/permute/cast) and unshard (transpose fp16
scores^T back to (sq, sk) f32, sum the 4 per-batch output-projection partials).
"""

import numpy as np

B, S, D = 2, 2048, 512
NUM_HEADS, HEAD_DIM = 8, 64
P = 128          # partitions
NT = 512         # free-dim tile (one PSUM bank of f32)
NT2 = 1024       # doubled free-dim tile for ACT exp ops
NKC = D // P     # 4 contraction chunks for projections
NSK = S // P     # 16 sk chunks
NSK2 = NSK // 2  # 8 pairs of sk chunks
NSQ = S // NT    # 4 sq tiles
VW = 2 * (HEAD_DIM + 1)  # 130: v columns per sk chunk (2 heads + 2 ones cols)

_CACHE = {}


def _build_nc():
    import concourse.tile as tile
    import concourse.mybir as mybir
    from concourse import bacc
    from contextlib import ExitStack

    f32 = mybir.dt.float32
    f32r = mybir.dt.float32r
    f16 = mybir.dt.float16
    Exp = mybir.ActivationFunctionType.Exp

    nc = bacc.Bacc("TRN2", target_bir_lowering=False, debug=False, num_devices=8)

    xqt = nc.dram_tensor("xqt", [D, S], f16, kind="ExternalInput").ap()
    xkt = nc.dram_tensor("xkt", [D, S], f16, kind="ExternalInput").ap()
    xvt = nc.dram_tensor("xvt", [D, S], f16, kind="ExternalInput").ap()
    wpack = nc.dram_tensor("wpack", [D, 5 * P], f16, kind="ExternalInput").ap()
    wo = nc.dram_tensor("wo", [P, D], f16, kind="ExternalInput").ap()
    cs = nc.dram_tensor("cs", [P, 2 * S], f16, kind="ExternalInput").ap()

    s_out = [
        nc.dram_tensor(f"s{h}", [S, S], f16, kind="ExternalOutput").ap()
        for h in range(2)
    ]
    p_out = nc.dram_tensor("po", [S, D], f16, kind="ExternalOutput").ap()

    with tile.TileContext(nc) as tc:
        with ExitStack() as ctx:
            consts = ctx.enter_context(tc.tile_pool(name="consts", bufs=1))
            xpool = ctx.enter_context(tc.tile_pool(name="xp", bufs=10))
            qkpool = ctx.enter_context(tc.tile_pool(name="qk", bufs=1))
            exps = ctx.enter_context(tc.tile_pool(name="exps", bufs=20))
            norms = ctx.enter_context(tc.tile_pool(name="norms", bufs=2))
            small = ctx.enter_context(tc.tile_pool(name="small", bufs=2))
            stage = ctx.enter_context(tc.tile_pool(name="stage", bufs=3))
            ps_l = ctx.enter_context(tc.tile_pool(name="psl", bufs=2, space="PSUM"))
            ps_e = ctx.enter_context(tc.tile_pool(name="pse", bufs=2, space="PSUM"))
            ps_m = ctx.enter_context(tc.tile_pool(name="psm", bufs=2, space="PSUM"))

            # warm the ACT exp table while DMAs stream in
            warm = consts.tile([1, 8], f32, tag="warm")
            nc.vector.memset(warm, 0.0)
            warm16 = consts.tile([1, 8], f16, tag="warm16")
            nc.scalar.activation(warm16, warm, Exp, scale=1.0)

            # ---- constants (one packed DMA each; X inputs on the ACT queue) ----
            WPW = 5 * P
            w_pack_sb = consts.tile([P, NKC * WPW], f16, tag="wpk")
            nc.sync.dma_start(
                out=w_pack_sb.rearrange("p (kc w) -> p kc w", w=WPW),
                in_=wpack.rearrange("(kc p) w -> p kc w", p=P),
            )
            w_off = {"wq": 0, "wk": P, "wq2": 2 * P, "wk2": 3 * P, "wv": 4 * P}

            def w_sl(name, kc):
                return w_pack_sb[:, kc * WPW + w_off[name]:kc * WPW + w_off[name] + P]

            wo_sb = [None, None]
            for h in range(2):
                t = consts.tile([HEAD_DIM, D], f16, tag=f"wo{h}", name=f"wo{h}")
                nc.sync.dma_start(
                    out=t, in_=wo[h * HEAD_DIM:(h + 1) * HEAD_DIM, :]
                )
                wo_sb[h] = t
            cs_sb = consts.tile([P, 2 * S], f16, tag="cs")
            nc.sync.dma_start(out=cs_sb, in_=cs)
            cos_sb = cs_sb[:, 0:S]
            sin_sb = cs_sb[:, S:2 * S]
            ones_sb = consts.tile([1, P], f16, tag="ones")
            nc.vector.memset(ones_sb, 1.0)

            x_sb = {}
            for name, dram, eng in (
                ("xk", xkt, nc.scalar), ("xq", xqt, nc.scalar), ("xv", xvt, nc.sync)
            ):
                t = consts.tile([P, NKC * S], f16, tag=f"x_{name}")
                for kc in range(NKC):
                    eng.dma_start(
                        out=t[:, kc * S:(kc + 1) * S],
                        in_=dram[kc * P:(kc + 1) * P, :],
                    )
                x_sb[name] = t

            def x_sl(name, kc, lo, hi):
                return x_sb[name][:, kc * S + lo:kc * S + hi]

            # ---- phase 1: projections + rope ----
            qT = qkpool.tile([P, S], f16, tag="qT")
            kT = qkpool.tile([P, S], f16, tag="kT")
            v_all = qkpool.tile([P, NSK * VW], f16, tag="v")
            # ones columns of v (cols 64 and 129 of each 130-block)
            nc.vector.memset(
                v_all.rearrange("p (c t) -> p c t", t=HEAD_DIM + 1)[:, :, HEAD_DIM:],
                1.0,
            )

            def proj_rope(wname, xname, dst, ntile):
                sl = slice(ntile * NT, (ntile + 1) * NT)
                py = ps_m.tile([P, NT], f32, tag="psm")
                py2 = ps_m.tile([P, NT], f32, tag="psm")
                for kc in range(NKC):
                    nc.tensor.matmul(
                        py,
                        lhsT=w_sl(wname, kc),
                        rhs=x_sl(xname, kc, ntile * NT, (ntile + 1) * NT),
                        start=(kc == 0),
                        stop=(kc == NKC - 1),
                    )
                for kc in range(NKC):
                    nc.tensor.matmul(
                        py2,
                        lhsT=w_sl(wname + "2", kc),
                        rhs=x_sl(xname, kc, ntile * NT, (ntile + 1) * NT),
                        start=(kc == 0),
                        stop=(kc == NKC - 1),
                    )
                # rope: dst = y*cos + y_swapped*(signed sin), all aligned
                t = small.tile([P, NT], f32, tag="ropet", bufs=3)
                u = small.tile([P, NT], f32, tag="ropeu", bufs=3)
                nc.vector.tensor_mul(t, py, cos_sb[:, sl])
                nc.vector.tensor_mul(u, py2, sin_sb[:, sl])
                nc.gpsimd.tensor_add(dst[:, sl], t, u)

            for ntile in range(NSQ):
                proj_rope("wk", "xk", kT, ntile)
            proj_rope("wq", "xq", qT, 0)

            # v: direct orientation (sk on partitions)
            for c in range(NSK):
                pv = ps_m.tile([P, P], f32, tag="psm")
                for kc in range(NKC):
                    nc.tensor.matmul(
                        pv,
                        lhsT=x_sl("xv", kc, c * P, (c + 1) * P),
                        rhs=w_sl("wv", kc),
                        start=(kc == 0),
                        stop=(kc == NKC - 1),
                    )
                nc.vector.tensor_copy(
                    v_all[:, c * VW:c * VW + HEAD_DIM], pv[:, 0:HEAD_DIM]
                )
                nc.vector.tensor_copy(
                    v_all[:, c * VW + HEAD_DIM + 1:c * VW + 2 * HEAD_DIM + 1],
                    pv[:, HEAD_DIM:2 * HEAD_DIM],
                )

            # ---- phase 2: attention, interleaving heads per sq tile ----
            e_sb = [
                qkpool.tile([HEAD_DIM, S], f16, tag=f"e{h}", name=f"e{h}")
                for h in range(2)
            ]
            def wo_proj(sq_t):
                stq = stage.tile([P, NSQ * D], f16, tag="st")
                for ci in range(NT // P):
                    sc = sq_t * (NT // P) + ci
                    po_ps = ps_m.tile([P, D], f32, tag="psm")
                    nc.tensor.matmul(
                        po_ps,
                        lhsT=e_sb[0][:, sc * P:(sc + 1) * P],
                        rhs=wo_sb[0],
                        start=True,
                        stop=False,
                    )
                    nc.tensor.matmul(
                        po_ps,
                        lhsT=e_sb[1][:, sc * P:(sc + 1) * P],
                        rhs=wo_sb[1],
                        start=False,
                        stop=True,
                    )
                    nc.vector.tensor_copy(stq[:, ci * D:(ci + 1) * D], po_ps)
                nc.sync.dma_start(
                    out=p_out.rearrange(
                        "(t ci p) d -> p t ci d", p=P, ci=NSQ
                    )[:, sq_t:sq_t + 1, :, :],
                    in_=stq.rearrange("p (o ci d) -> p o ci d", o=1, d=D),
                )

            for sq_t in range(NSQ):
                if sq_t > 0:
                    proj_rope("wq", "xq", qT, sq_t)
                sq = slice(sq_t * NT, (sq_t + 1) * NT)
                for h in range(2):
                    po = 64 * h
                    vo = (HEAD_DIM + 1) * h
                    pe_acc = ps_e.tile([HEAD_DIM + 1, NT], f32, tag="pse")
                    etiles = []
                    for c2 in range(NSK2):
                        pl = ps_l.tile([P, NT2], f32, tag="psl")
                        for half in range(2):
                            c = 2 * c2 + half
                            nc.tensor.matmul(
                                pl[:, half * NT:(half + 1) * NT],
                                lhsT=kT[po:po + 64, c * P:(c + 1) * P],
                                rhs=qT[po:po + 64, sq],
                                start=True,
                                stop=True,
                            )
                        e = exps.tile([P, NT2], f16, tag="exp")
                        nc.scalar.activation(e, pl, Exp, scale=0.125)
                        etiles.append(e)
                    for c2 in range(NSK2):
                        for half in range(2):
                            c = 2 * c2 + half
                            nc.tensor.matmul(
                                pe_acc[0:HEAD_DIM + 1, :],
                                lhsT=v_all[
                                    :, c * VW + vo:c * VW + vo + HEAD_DIM + 1
                                ],
                                rhs=etiles[c2][:, half * NT:(half + 1) * NT],
                                start=(c == 0),
                                stop=(c == NSK - 1),
                            )
                    # denominators -> reciprocal -> fp16 row
                    dn = small.tile([1, NT], f32, tag="dn")
                    nc.vector.tensor_copy(dn, pe_acc[HEAD_DIM:HEAD_DIM + 1, :])
                    rc = small.tile([1, NT], f32, tag="rc")
                    nc.vector.reciprocal_approx_fast(out=rc, in_=dn)
                    rc16 = small.tile([1, NT], f16, tag="rc16")
                    nc.vector.tensor_copy(rc16, rc)
                    # broadcast across 128 partitions via rank-1 matmul
                    pb = ps_m.tile([P, NT], f32, tag="psm")
                    nc.tensor.matmul(pb, lhsT=ones_sb, rhs=rc16, start=True, stop=True)
                    bc = small.tile([P, NT], f16, tag="bc")
                    nc.vector.tensor_copy(bc, pb)
                    # normalize scores + batched DMA out
                    nall = norms.tile([P, NSK * NT], f16, tag="norm")
                    for c2 in range(NSK2):
                        for half in range(2):
                            c = 2 * c2 + half
                            nc.vector.tensor_mul(
                                nall[:, c * NT:(c + 1) * NT],
                                etiles[c2][:, half * NT:(half + 1) * NT],
                                bc,
                            )
                    last = (sq_t == NSQ - 1) and (h == 1)
                    if last:
                        sv = s_out[h].rearrange(
                            "(g c p) (t x) -> p g c t x", p=P, c=NSQ, x=NT
                        )
                        nv = nall.rearrange(
                            "p (g c o x) -> p g c o x", c=NSQ, o=1, x=NT
                        )
                        for g in range(NSK // NSQ):
                            nc.sync.dma_start(
                                out=sv[:, g:g + 1, :, sq_t:sq_t + 1, :],
                                in_=nv[:, g:g + 1, :, :, :],
                            )
                    else:
                        nc.sync.dma_start(
                            out=s_out[h].rearrange(
                                "(c p) (t x) -> p c t x", p=P, x=NT
                            )[:, :, sq_t:sq_t + 1, :],
                            in_=nall.rearrange("p (c o x) -> p c o x", o=1, x=NT),
                        )
                    # normalize embeds slice
                    nc.vector.tensor_mul(
                        e_sb[h][:, sq], pe_acc[0:HEAD_DIM, :], bc[0:HEAD_DIM, :]
                    )
                # deferred output projection of the previous sq tile
                if sq_t > 0:
                    wo_proj(sq_t - 1)
            wo_proj(NSQ - 1)

    nc.compile()
    return nc


def _get_nc():
    if "nc" not in _CACHE:
        _CACHE["nc"] = _build_nc()
    return _CACHE["nc"]


def _rope_tables(hp):
    """cos/sin tiles (128, S) f32 for head pair (hp, hp+4), doubled rows;
    sin rows carry the rope signs (-sin on top half, +sin on bottom)."""
    freqs = np.exp(
        np.arange(0, D, 2, dtype=np.float32) * np.float32(-np.log(10000.0) / D)
    ).astype(np.float32)
    sel = freqs[64 * hp:64 * hp + 64]
    pos = np.arange(S, dtype=np.float32)
    ang = pos[:, None] * sel[None, :]          # (S, 64) f32
    ct = np.cos(ang).astype(np.float32).T      # (64, S)
    st = np.sin(ang).astype(np.float32).T
    c = np.ascontiguousarray(np.concatenate([ct, ct], axis=0))
    s = np.ascontiguousarray(np.concatenate([-st, st], axis=0))
    return c, s


def _transpose_f16_to_f32(a):
    """(S, S) fp16 -> transposed (S, S) fp32, cache-blocked."""
    out = np.empty((S, S), np.float32)
    blk = 128
    for j in range(0, S, blk):
        out[:, j:j + blk] = a[j:j + blk, :].T.astype(np.float32)
    return out


def _numpy_reference(querys, keys, values, attn_mask, Wq, Wk, Wv, Wo):
    def rope(x):
        b, s, d = x.shape
        pos = np.arange(s, dtype=np.float32)[:, None]
        freqs = np.exp(
            np.arange(0, d, 2, dtype=np.float32) * np.float32(-np.log(10000.0) / d)
        )
        ang = pos * freqs
        cos = np.cos(ang)[None].astype(np.float32)
        sin = np.sin(ang)[None].astype(np.float32)
        xe, xo = x[..., 0::2], x[..., 1::2]
        return np.concatenate([xe * cos - xo * sin, xo * cos + xe * sin], axis=-1)

    def split_heads(x):
        b, s, d = x.shape
        return x.reshape(b, s, NUM_HEADS, HEAD_DIM).transpose(0, 2, 1, 3)

    q = split_heads(rope(querys @ Wq.T))
    k = split_heads(rope(keys @ Wk.T))
    v = split_heads(values @ Wv.T)
    logits = np.einsum("bhqd,bhkd->bhqk", q, k)
    logits = logits + attn_mask[:, :, :S, :S] * np.float32(-1e9)
    x = logits / np.float32(np.sqrt(HEAD_DIM))
    x = x - x.max(axis=-1, keepdims=True)
    ex = np.exp(x)
    attn = ex / ex.sum(axis=-1, keepdims=True)
    emb = np.einsum("bhqk,bhkd->bhqd", attn, v)
    b, h, s, hd = emb.shape
    emb = emb.transpose(0, 2, 1, 3).reshape(b, s, h * hd) @ Wo.T
    return emb.astype(np.float32), attn.astype(np.float32)


def kernel(querys, keys, values, attn_mask, Wq, Wk, Wv, Wo):
    querys = np.asarray(querys, dtype=np.float32)
    keys = np.asarray(keys, dtype=np.float32)
    values = np.asarray(values, dtype=np.float32)
    attn_mask = np.asarray(attn_mask, dtype=np.float32)
    Wq = np.asarray(Wq, dtype=np.float32)
    Wk = np.asarray(Wk, dtype=np.float32)
    Wv = np.asarray(Wv, dtype=np.float32)
    Wo = np.asarray(Wo, dtype=np.float32)

    if np.any(attn_mask):
        # The device kernel hardcodes a zero mask (spec fill: zeros).
        return _numpy_reference(
            querys, keys, values, attn_mask, Wq, Wk, Wv, Wo
        )

    from concourse.bass_utils import run_bass_kernel_spmd

    nc = _get_nc()

    xt = {}
    for b in range(B):
        xt[("q", b)] = np.ascontiguousarray(querys[b].T).astype(np.float16)
        xt[("k", b)] = np.ascontiguousarray(keys[b].T).astype(np.float16)
        xt[("v", b)] = np.ascontiguousarray(values[b].T).astype(np.float16)

    in_maps = []
    for c in range(8):
        b, hp = divmod(c, 4)
        perm_qk = np.concatenate(
            [128 * hp + 2 * np.arange(64), 128 * hp + 2 * np.arange(64) + 1]
        )
        perm_qk2 = np.concatenate(
            [128 * hp + 2 * np.arange(64) + 1, 128 * hp + 2 * np.arange(64)]
        )
        perm_v = np.concatenate(
            [64 * hp + np.arange(64), 256 + 64 * hp + np.arange(64)]
        )
        cos_t, sin_t = _rope_tables(hp)
        in_maps.append({
            "xqt": xt[("q", b)],
            "xkt": xt[("k", b)],
            "xvt": xt[("v", b)],
            "wpack": np.ascontiguousarray(np.concatenate([
                Wq[perm_qk, :].T, Wk[perm_qk, :].T,
                Wq[perm_qk2, :].T, Wk[perm_qk2, :].T,
                Wv[perm_v, :].T,
            ], axis=1)).astype(np.float16),
            "wo": np.ascontiguousarray(Wo[:, perm_v].T).astype(np.float16),
            "cs": np.ascontiguousarray(
                np.concatenate([cos_t, sin_t], axis=1)
            ).astype(np.float16),
        })

    res = run_bass_kernel_spmd(nc, in_maps, core_ids=list(range(8)))

    attn_scores = np.empty((B, NUM_HEADS, S, S), np.float32)
    embeds = np.zeros((B, S, D), np.float32)
    for c in range(8):
        b, hp = divmod(c, 4)
        attn_scores[b, hp] = _transpose_f16_to_f32(res.results[c]["s0"])
        attn_scores[b, hp + 4] = _transpose_f16_to_f32(res.results[c]["s1"])
        embeds[b] += res.results[c]["po"].astype(np.float32)

    return embeds, attn_scores


# revision 12
# speedup vs baseline: 1.3089x; 1.2414x over previous
"""Trainium2 Bass kernel for nn_MultiHeadAttention (B=2, S=2048, D=512, H=8).

Sharding: 8 cores = 2 batches x 4 head-pairs. Core c handles batch b=c//4 and
heads {hp, hp+4} with hp=c%4 (the pair shares rope frequencies and the same
128 columns of the QK projections, so the projection work is shared).

Per-core device pipeline (all seq-transposed, "layout B" — sk on partitions):
  f32r projections (X^T @ W slices, plus half-swapped copies so rope is
  partition-aligned) -> rope on DVE (fp16 q^T/k^T)
  -> logits^T = k @ q^T per head (fp16 matmuls, fp32 PSUM)
  -> exp on ScalarE (scale=1/8 folded in, fp16 out, FD=1024 ops)
  -> V-matmul with a ones-column appended to v => embeds^T and softmax
     denominators accumulate in one PSUM tile
  -> reciprocal + PE rank-1 broadcast -> DVE normalize -> fp16 scores^T out
  -> output projection partials (E^T @ Wo^T slice) -> fp16 partial out
Host side: shard prep (transpose/permute/cast) and unshard (transpose fp16
scores^T back to (sq, sk) f32, sum the 4 per-batch output-projection partials).
"""

import numpy as np

B, S, D = 2, 2048, 512
NUM_HEADS, HEAD_DIM = 8, 64
P = 128          # partitions
NT = 512         # free-dim tile (one PSUM bank of f32)
NT2 = 1024       # doubled free-dim tile for ACT exp ops
NKC = D // P     # 4 contraction chunks for projections
NSK = S // P     # 16 sk chunks
NSK2 = NSK // 2  # 8 pairs of sk chunks
NSQ = S // NT    # 4 sq tiles
VW = 2 * (HEAD_DIM + 1)  # 130: v columns per sk chunk (2 heads + 2 ones cols)

_CACHE = {}


def _build_nc():
    import concourse.tile as tile
    import concourse.mybir as mybir
    from concourse import bacc
    from contextlib import ExitStack

    f32 = mybir.dt.float32
    f32r = mybir.dt.float32r
    f16 = mybir.dt.float16
    Exp = mybir.ActivationFunctionType.Exp

    nc = bacc.Bacc("TRN2", target_bir_lowering=False, debug=False, num_devices=8)

    xqt = nc.dram_tensor("xqt", [D, S], f16, kind="ExternalInput").ap()
    xkt = nc.dram_tensor("xkt", [D, S], f16, kind="ExternalInput").ap()
    xvt = nc.dram_tensor("xvt", [D, S], f16, kind="ExternalInput").ap()
    wpack = nc.dram_tensor("wpack", [D, 5 * P], f16, kind="ExternalInput").ap()
    wo = nc.dram_tensor("wo", [P, D], f16, kind="ExternalInput").ap()
    cs = nc.dram_tensor("cs", [P, 2 * S], f16, kind="ExternalInput").ap()

    s_out = [
        nc.dram_tensor(f"s{h}", [S, S], f16, kind="ExternalOutput").ap()
        for h in range(2)
    ]
    p_out = nc.dram_tensor("po", [S, D], f16, kind="ExternalOutput").ap()

    with tile.TileContext(nc) as tc:
        with ExitStack() as ctx:
            consts = ctx.enter_context(tc.tile_pool(name="consts", bufs=1))
            xpool = ctx.enter_context(tc.tile_pool(name="xp", bufs=10))
            qkpool = ctx.enter_context(tc.tile_pool(name="qk", bufs=1))
            exps = ctx.enter_context(tc.tile_pool(name="exps", bufs=20))
            norms = ctx.enter_context(tc.tile_pool(name="norms", bufs=2))
            small = ctx.enter_context(tc.tile_pool(name="small", bufs=2))
            stage = ctx.enter_context(tc.tile_pool(name="stage", bufs=3))
            ps_l = ctx.enter_context(tc.tile_pool(name="psl", bufs=2, space="PSUM"))
            ps_e = ctx.enter_context(tc.tile_pool(name="pse", bufs=2, space="PSUM"))
            ps_m = ctx.enter_context(tc.tile_pool(name="psm", bufs=2, space="PSUM"))

            # warm the ACT exp table while DMAs stream in
            warm = consts.tile([1, 8], f32, tag="warm")
            nc.vector.memset(warm, 0.0)
            warm16 = consts.tile([1, 8], f16, tag="warm16")
            nc.scalar.activation(warm16, warm, Exp, scale=1.0)

            # ---- constants (one packed DMA each; X inputs on the ACT queue) ----
            WPW = 5 * P
            w_pack_sb = consts.tile([P, NKC * WPW], f16, tag="wpk")
            nc.sync.dma_start(
                out=w_pack_sb.rearrange("p (kc w) -> p kc w", w=WPW),
                in_=wpack.rearrange("(kc p) w -> p kc w", p=P),
            )
            w_off = {"wq": 0, "wk": P, "wq2": 2 * P, "wk2": 3 * P, "wv": 4 * P}

            def w_sl(name, kc):
                return w_pack_sb[:, kc * WPW + w_off[name]:kc * WPW + w_off[name] + P]

            wo_sb = [None, None]
            for h in range(2):
                t = consts.tile([HEAD_DIM, D], f16, tag=f"wo{h}", name=f"wo{h}")
                nc.sync.dma_start(
                    out=t, in_=wo[h * HEAD_DIM:(h + 1) * HEAD_DIM, :]
                )
                wo_sb[h] = t
            cs_sb = consts.tile([P, 2 * S], f16, tag="cs")
            nc.sync.dma_start(out=cs_sb, in_=cs)
            cos_sb = cs_sb[:, 0:S]
            sin_sb = cs_sb[:, S:2 * S]
            ones_sb = consts.tile([1, P], f16, tag="ones")
            nc.vector.memset(ones_sb, 1.0)

            x_sb = {}
            for name, dram, eng in (
                ("xk", xkt, nc.scalar), ("xq", xqt, nc.scalar), ("xv", xvt, nc.sync)
            ):
                t = consts.tile([P, NKC * S], f16, tag=f"x_{name}")
                for kc in range(NKC):
                    eng.dma_start(
                        out=t[:, kc * S:(kc + 1) * S],
                        in_=dram[kc * P:(kc + 1) * P, :],
                    )
                x_sb[name] = t

            def x_sl(name, kc, lo, hi):
                return x_sb[name][:, kc * S + lo:kc * S + hi]

            # ---- phase 1: projections + rope ----
            qT = qkpool.tile([P, S], f16, tag="qT")
            kT = qkpool.tile([P, S], f16, tag="kT")
            v_all = qkpool.tile([P, NSK * VW], f16, tag="v")
            # ones columns of v (cols 64 and 129 of each 130-block)
            nc.vector.memset(
                v_all.rearrange("p (c t) -> p c t", t=HEAD_DIM + 1)[:, :, HEAD_DIM:],
                1.0,
            )

            def proj_rope(wname, xname, dst, ntile):
                sl = slice(ntile * NT, (ntile + 1) * NT)
                py = ps_m.tile([P, NT], f32, tag="psm")
                py2 = ps_m.tile([P, NT], f32, tag="psm")
                for kc in range(NKC):
                    nc.tensor.matmul(
                        py,
                        lhsT=w_sl(wname, kc),
                        rhs=x_sl(xname, kc, ntile * NT, (ntile + 1) * NT),
                        start=(kc == 0),
                        stop=(kc == NKC - 1),
                    )
                for kc in range(NKC):
                    nc.tensor.matmul(
                        py2,
                        lhsT=w_sl(wname + "2", kc),
                        rhs=x_sl(xname, kc, ntile * NT, (ntile + 1) * NT),
                        start=(kc == 0),
                        stop=(kc == NKC - 1),
                    )
                # rope: dst = y*cos + y_swapped*(signed sin), all aligned
                t = small.tile([P, NT], f32, tag="ropet", bufs=3)
                u = small.tile([P, NT], f32, tag="ropeu", bufs=3)
                nc.vector.tensor_mul(t, py, cos_sb[:, sl])
                nc.vector.tensor_mul(u, py2, sin_sb[:, sl])
                nc.gpsimd.tensor_add(dst[:, sl], t, u)

            for ntile in range(NSQ):
                proj_rope("wk", "xk", kT, ntile)
            proj_rope("wq", "xq", qT, 0)

            # v: direct orientation (sk on partitions)
            for c in range(NSK):
                pv = ps_m.tile([P, P], f32, tag="psm")
                for kc in range(NKC):
                    nc.tensor.matmul(
                        pv,
                        lhsT=x_sl("xv", kc, c * P, (c + 1) * P),
                        rhs=w_sl("wv", kc),
                        start=(kc == 0),
                        stop=(kc == NKC - 1),
                    )
                nc.scalar.copy(
                    v_all[:, c * VW:c * VW + HEAD_DIM], pv[:, 0:HEAD_DIM]
                )
                nc.scalar.copy(
                    v_all[:, c * VW + HEAD_DIM + 1:c * VW + 2 * HEAD_DIM + 1],
                    pv[:, HEAD_DIM:2 * HEAD_DIM],
                )

            # ---- phase 2: attention, interleaving heads per sq tile ----
            e_sb = [
                qkpool.tile([HEAD_DIM, S], f16, tag=f"e{h}", name=f"e{h}")
                for h in range(2)
            ]
            def wo_proj(sq_t):
                stq = stage.tile([P, NSQ * D], f16, tag="st")
                for ci in range(NT // P):
                    sc = sq_t * (NT // P) + ci
                    po_ps = ps_e.tile([P, D], f32, tag="pse")
                    nc.tensor.matmul(
                        po_ps,
                        lhsT=e_sb[0][:, sc * P:(sc + 1) * P],
                        rhs=wo_sb[0],
                        start=True,
                        stop=False,
                    )
                    nc.tensor.matmul(
                        po_ps,
                        lhsT=e_sb[1][:, sc * P:(sc + 1) * P],
                        rhs=wo_sb[1],
                        start=False,
                        stop=True,
                    )
                    nc.vector.tensor_copy(stq[:, ci * D:(ci + 1) * D], po_ps)
                nc.sync.dma_start(
                    out=p_out.rearrange(
                        "(t ci p) d -> p t ci d", p=P, ci=NSQ
                    )[:, sq_t:sq_t + 1, :, :],
                    in_=stq.rearrange("p (o ci d) -> p o ci d", o=1, d=D),
                )

            for sq_t in range(NSQ):
                if sq_t + 1 < NSQ:
                    proj_rope("wq", "xq", qT, sq_t + 1)
                sq = slice(sq_t * NT, (sq_t + 1) * NT)
                for h in range(2):
                    po = 64 * h
                    vo = (HEAD_DIM + 1) * h
                    pe_acc = ps_e.tile([HEAD_DIM + 1, NT], f32, tag="pse")
                    etiles = []
                    for c2 in range(NSK2):
                        pl = ps_l.tile([P, NT2], f32, tag="psl")
                        for half in range(2):
                            c = 2 * c2 + half
                            nc.tensor.matmul(
                                pl[:, half * NT:(half + 1) * NT],
                                lhsT=kT[po:po + 64, c * P:(c + 1) * P],
                                rhs=qT[po:po + 64, sq],
                                start=True,
                                stop=True,
                            )
                        e = exps.tile([P, NT2], f16, tag="exp")
                        nc.scalar.activation(e, pl, Exp, scale=0.125)
                        etiles.append(e)
                    for c2 in range(NSK2):
                        for half in range(2):
                            c = 2 * c2 + half
                            nc.tensor.matmul(
                                pe_acc[0:HEAD_DIM + 1, :],
                                lhsT=v_all[
                                    :, c * VW + vo:c * VW + vo + HEAD_DIM + 1
                                ],
                                rhs=etiles[c2][:, half * NT:(half + 1) * NT],
                                start=(c == 0),
                                stop=(c == NSK - 1),
                            )
                    # denominators -> reciprocal -> fp16 row
                    dn = small.tile([1, NT], f32, tag="dn")
                    nc.scalar.copy(dn, pe_acc[HEAD_DIM:HEAD_DIM + 1, :])
                    rc = small.tile([1, NT], f32, tag="rc")
                    nc.vector.reciprocal_approx_fast(out=rc, in_=dn)
                    rc16 = small.tile([1, NT], f16, tag="rc16")
                    nc.vector.tensor_copy(rc16, rc)
                    # broadcast across 128 partitions via rank-1 matmul
                    pb = ps_m.tile([P, NT], f32, tag="psm")
                    nc.tensor.matmul(pb, lhsT=ones_sb, rhs=rc16, start=True, stop=True)
                    bc = small.tile([P, NT], f16, tag="bc")
                    nc.vector.tensor_copy(bc, pb)
                    # normalize scores + batched DMA out
                    nall = norms.tile([P, NSK * NT], f16, tag="norm")
                    for c2 in range(NSK2):
                        for half in range(2):
                            c = 2 * c2 + half
                            nc.vector.tensor_mul(
                                nall[:, c * NT:(c + 1) * NT],
                                etiles[c2][:, half * NT:(half + 1) * NT],
                                bc,
                            )
                    last = (sq_t == NSQ - 1) and (h == 1)
                    if last:
                        sv = s_out[h].rearrange(
                            "(g c p) (t x) -> p g c t x", p=P, c=NSQ, x=NT
                        )
                        nv = nall.rearrange(
                            "p (g c o x) -> p g c o x", c=NSQ, o=1, x=NT
                        )
                        for g in range(NSK // NSQ):
                            nc.sync.dma_start(
                                out=sv[:, g:g + 1, :, sq_t:sq_t + 1, :],
                                in_=nv[:, g:g + 1, :, :, :],
                            )
                    else:
                        nc.sync.dma_start(
                            out=s_out[h].rearrange(
                                "(c p) (t x) -> p c t x", p=P, x=NT
                            )[:, :, sq_t:sq_t + 1, :],
                            in_=nall.rearrange("p (c o x) -> p c o x", o=1, x=NT),
                        )
                    # normalize embeds slice
                    nc.vector.tensor_mul(
                        e_sb[h][:, sq], pe_acc[0:HEAD_DIM, :], bc[0:HEAD_DIM, :]
                    )
                # deferred output projection of the previous sq tile
                if sq_t > 0:
                    wo_proj(sq_t - 1)
            wo_proj(NSQ - 1)

    nc.compile()
    return nc


def _get_nc():
    if "nc" not in _CACHE:
        _CACHE["nc"] = _build_nc()
    return _CACHE["nc"]


def _rope_tables(hp):
    """cos/sin tiles (128, S) f32 for head pair (hp, hp+4), doubled rows;
    sin rows carry the rope signs (-sin on top half, +sin on bottom)."""
    freqs = np.exp(
        np.arange(0, D, 2, dtype=np.float32) * np.float32(-np.log(10000.0) / D)
    ).astype(np.float32)
    sel = freqs[64 * hp:64 * hp + 64]
    pos = np.arange(S, dtype=np.float32)
    ang = pos[:, None] * sel[None, :]          # (S, 64) f32
    ct = np.cos(ang).astype(np.float32).T      # (64, S)
    st = np.sin(ang).astype(np.float32).T
    c = np.ascontiguousarray(np.concatenate([ct, ct], axis=0))
    s = np.ascontiguousarray(np.concatenate([-st, st], axis=0))
    return c, s


def _transpose_f16_to_f32(a):
    """(S, S) fp16 -> transposed (S, S) fp32, cache-blocked."""
    out = np.empty((S, S), np.float32)
    blk = 128
    for j in range(0, S, blk):
        out[:, j:j + blk] = a[j:j + blk, :].T.astype(np.float32)
    return out


def _numpy_reference(querys, keys, values, attn_mask, Wq, Wk, Wv, Wo):
    def rope(x):
        b, s, d = x.shape
        pos = np.arange(s, dtype=np.float32)[:, None]
        freqs = np.exp(
            np.arange(0, d, 2, dtype=np.float32) * np.float32(-np.log(10000.0) / d)
        )
        ang = pos * freqs
        cos = np.cos(ang)[None].astype(np.float32)
        sin = np.sin(ang)[None].astype(np.float32)
        xe, xo = x[..., 0::2], x[..., 1::2]
        return np.concatenate([xe * cos - xo * sin, xo * cos + xe * sin], axis=-1)

    def split_heads(x):
        b, s, d = x.shape
        return x.reshape(b, s, NUM_HEADS, HEAD_DIM).transpose(0, 2, 1, 3)

    q = split_heads(rope(querys @ Wq.T))
    k = split_heads(rope(keys @ Wk.T))
    v = split_heads(values @ Wv.T)
    logits = np.einsum("bhqd,bhkd->bhqk", q, k)
    logits = logits + attn_mask[:, :, :S, :S] * np.float32(-1e9)
    x = logits / np.float32(np.sqrt(HEAD_DIM))
    x = x - x.max(axis=-1, keepdims=True)
    ex = np.exp(x)
    attn = ex / ex.sum(axis=-1, keepdims=True)
    emb = np.einsum("bhqk,bhkd->bhqd", attn, v)
    b, h, s, hd = emb.shape
    emb = emb.transpose(0, 2, 1, 3).reshape(b, s, h * hd) @ Wo.T
    return emb.astype(np.float32), attn.astype(np.float32)


def kernel(querys, keys, values, attn_mask, Wq, Wk, Wv, Wo):
    querys = np.asarray(querys, dtype=np.float32)
    keys = np.asarray(keys, dtype=np.float32)
    values = np.asarray(values, dtype=np.float32)
    attn_mask = np.asarray(attn_mask, dtype=np.float32)
    Wq = np.asarray(Wq, dtype=np.float32)
    Wk = np.asarray(Wk, dtype=np.float32)
    Wv = np.asarray(Wv, dtype=np.float32)
    Wo = np.asarray(Wo, dtype=np.float32)

    if np.any(attn_mask):
        # The device kernel hardcodes a zero mask (spec fill: zeros).
        return _numpy_reference(
            querys, keys, values, attn_mask, Wq, Wk, Wv, Wo
        )

    from concourse.bass_utils import run_bass_kernel_spmd

    nc = _get_nc()

    xt = {}
    for b in range(B):
        xt[("q", b)] = np.ascontiguousarray(querys[b].T).astype(np.float16)
        xt[("k", b)] = np.ascontiguousarray(keys[b].T).astype(np.float16)
        xt[("v", b)] = np.ascontiguousarray(values[b].T).astype(np.float16)

    in_maps = []
    for c in range(8):
        b, hp = divmod(c, 4)
        perm_qk = np.concatenate(
            [128 * hp + 2 * np.arange(64), 128 * hp + 2 * np.arange(64) + 1]
        )
        perm_qk2 = np.concatenate(
            [128 * hp + 2 * np.arange(64) + 1, 128 * hp + 2 * np.arange(64)]
        )
        perm_v = np.concatenate(
            [64 * hp + np.arange(64), 256 + 64 * hp + np.arange(64)]
        )
        cos_t, sin_t = _rope_tables(hp)
        in_maps.append({
            "xqt": xt[("q", b)],
            "xkt": xt[("k", b)],
            "xvt": xt[("v", b)],
            "wpack": np.ascontiguousarray(np.concatenate([
                Wq[perm_qk, :].T, Wk[perm_qk, :].T,
                Wq[perm_qk2, :].T, Wk[perm_qk2, :].T,
                Wv[perm_v, :].T,
            ], axis=1)).astype(np.float16),
            "wo": np.ascontiguousarray(Wo[:, perm_v].T).astype(np.float16),
            "cs": np.ascontiguousarray(
                np.concatenate([cos_t, sin_t], axis=1)
            ).astype(np.float16),
        })

    res = run_bass_kernel_spmd(nc, in_maps, core_ids=list(range(8)))

    attn_scores = np.empty((B, NUM_HEADS, S, S), np.float32)
    embeds = np.zeros((B, S, D), np.float32)
    for c in range(8):
        b, hp = divmod(c, 4)
        attn_scores[b, hp] = _transpose_f16_to_f32(res.results[c]["s0"])
        attn_scores[b, hp + 4] = _transpose_f16_to_f32(res.results[c]["s1"])
        embeds[b] += res.results[c]["po"].astype(np.float32)

    return embeds, attn_scores
